# revision 96
# baseline (speedup 1.0000x reference)
"""Trainium2 Bass kernel for nn_CNN2 (time-lagged cross-correlation CNN).

Math note (exact algebraic identity, not an approximation):
  The reference computes Y = W @ ones(30, T), so every time-column of Y is
  r = W.sum(axis=1).  The full lagged cross-correlation is then
  S[lag] = count(lag) * r r^T, its trace is count(lag) * ||r||^2, so the
  per-lag trace-normalized matrix S_mean[lag] = r r^T / ||r||^2 is the SAME
  for every lag.  Hence mean-over-lags = r r^T/||r||^2 and var-over-lags = 0
  for ANY W and ANY T.  The kernel therefore computes
      Gm = 0.5*(r r^T/||r||^2 + 1),  Gv = 0.5
  followed by the CNN tail (conv 2->8 k4 p1, leaky 0.2, maxpool 8;
  conv 8->16 k2 p1, leaky, maxpool 4; linear 16->2), all on-device.

Implementation notes (latency-driven; the kernel is dominated by fixed
DMA/semaphore latencies, so the design minimizes the serial depth between
the W input DMA and the output DMA):
  - The conv1 input channels split linearly: Gm = M + 0.5*mask and
    Gv = 0.5*mask with M = 0.5*r r^T/||r||^2.  The mask part of
    conv1 (+ conv1 bias) does not depend on W at all, so it is precomputed
    on the host and injected into the conv1 PSUM banks by one
    identity-weight matmul per bank.
  - r never leaves the chip, and W^T ships from the host directly in
    fp16 (the image is fp16 anyway, so this costs no precision and
    removes an on-chip convert).  rp16f = ones^T @ W^T replicates r on
    16 PSUM partitions for the ||r||^2 chain (Activation square-accum ->
    DVE reciprocal), and eight tiny one-hot-lhsT fp16 matmuls against
    shifted column windows of W^T build BOTH shift-Hankels in one PSUM
    tile: QS16[4kh+kw, y] = r[y+kh-1], rp16h[4kh+kw, j] = r[j+kw-1].
    One DVE copy moves them to SBUF and two multi-partition DVE stt ops
    (1/||r||^2 fused as the per-lane scalar) build all SIXTEEN
    doubly-shifted rank-1 image copies R[4kh+kw, 32y+j] =
    image[y+kh, j+kw], so each conv1 bank is ONE K=16 fp16 matmul.
    This replaces the SBUF->SBUF fan-out DMA of the first version
    (~2.2us of descriptor/semaphore latency) with ~1us of overlapped
    PE+DVE work.
  - conv1 output rows are split into three PSUM banks of one 8-row pool
    band each, so each band's maxpool reduce (DVE) pipelines behind the
    next band's matmuls.
  - maxpool commutes with x -> leaky(x+b) (monotone, per-channel b): conv2
    bias is accumulated into PSUM via a bias-row x ones-row matmul, pooling
    runs directly on PSUM, and only the leaky remains on the pooled tensors.
  - the linear-layer bias is pre-accumulated into the output PSUM early
    (ones(1,1) x bias-row matmul), so after the last matmul the result only
    needs a PSUM->SBUF copy before the output DMA.
  - conv1 runs in fp16 (1 PE cycle/row; checked end-to-end error ~3e-4).
    A tiny ones x ones matmul with no input dependencies runs right after
    the entry barrier to start the PE clock-ramp tracker early, so the
    critical matmuls run at the ramped clock rate.

The computation is replicated on the 8 NeuronCores (it is far below one
core's capacity; a cross-core split would only add collective latency), and
core 0's output is returned.
"""

import numpy as np

N = 30

_CACHE = {}


def _build_nc():
    from contextlib import ExitStack

    import concourse.bass as bass
    import concourse.tile as tile
    from concourse import bacc, mybir

    f32 = mybir.dt.float32
    f16 = mybir.dt.float16
    ALU = mybir.AluOpType
    AX = mybir.AxisListType

    nc = bacc.Bacc("TRN2")

    wt_d = nc.dram_tensor("wt", [N, 15], f32, kind="ExternalInput")     # W^T f16
    wp_d = nc.dram_tensor("wpack", [16, 384], f32, kind="ExternalInput")
    out_d = nc.dram_tensor("out", [1, 2], f32, kind="ExternalOutput")

    with tile.TileContext(nc) as tc, ExitStack() as ctx:
        sb = ctx.enter_context(tc.tile_pool(name="sb", bufs=1))
        ps = ctx.enter_context(tc.tile_pool(name="ps", bufs=1, space="PSUM"))

        # Pool memsets, in priority order (the first ones get hoisted ahead
        # of the entry barrier): one1 feeds the PE warm-up matmul.
        one1 = sb.tile([1, 1], f32)
        nc.gpsimd.memset(one1, 1.0)
        wpack = sb.tile([16, 384], f32)
        nc.gpsimd.dma_start(out=wpack, in_=wp_d.ap())
        ones30x16 = sb.tile([N, 16], f16)
        nc.gpsimd.memset(ones30x16, 1.0)
        # One-hot selector lhsT blocks.  The 16 image lanes are indexed
        # p = 4*kh + kw.  eselq block kh (cols 16kh..16kh+15) has ones at
        # within-block cols 4kh+t (t=0..3) -> global cols 20kh+t: the kh-th
        # QS16 matmul writes lanes 4kh..4kh+3.  eselr block kw has ones at
        # within-block cols kw+4t -> global cols 17kw+4t: the kw-th rp16
        # matmul writes lanes kw, kw+4, kw+8, kw+12.
        eselq = sb.tile([N, 64], f16)
        nc.gpsimd.memset(eselq, 0.0)
        nc.gpsimd.memset(bass.AP(eselq.tensor, eselq.offset,
                                 [list(eselq.ap[0]), [20, 4], [1, 4]]), 1.0)
        eselr = sb.tile([N, 64], f16)
        nc.gpsimd.memset(eselr, 0.0)
        nc.gpsimd.memset(bass.AP(eselr.tensor, eselr.offset,
                                 [list(eselr.ap[0]), [17, 4], [4, 4]]), 1.0)

        wt = sb.tile([N, 15], f32)
        nc.sync.dma_start(out=wt, in_=wt_d.ap())
        wt16 = wt.bitcast(f16)                 # (30, 30) f16 W^T view

        w1r = wpack[0:16, 0:4].bitcast(f16)    # (16, 8): 0.5*w1[:,0] (kh kw, co)
        w2l = wpack[0:8, 16:48].bitcast(f16)   # conv2 weights (ci, (pos co))
        static8 = wpack[0:8, 80:368].bitcast(f16)   # (8, 576) mask-ch conv out
        i8 = wpack[0:8, 368:372].bitcast(f16)       # (8, 8) identity
        owt = wpack[0:16, 372:374]
        ob = wpack[0:1, 374:376]
        b2row = wpack[0:1, 376:384].bitcast(f16)    # (1, 16) f16

        # fp16 conv rhs tile: R[4kh+kw, 32y+j] = image[y+kh, j+kw] for
        # j in 0..23 -- each lane is the image shifted in BOTH kh and kw,
        # so conv1's rank-1 channel is ONE K=16 matmul per bank.  Columns
        # 24..31 of each 32-block are never read, so no border memset.
        R = sb.tile([16, 768], f16)
        oner = sb.tile([1, 16], f16)
        nc.gpsimd.memset(oner, 1.0)
        p1p = sb.tile([8, 25], f16)
        nc.gpsimd.memset(p1p, 0.0)

        # PE warm-up: no input deps, starts the clock-ramp tracker early.
        # It borrows ps3 (overwritten below by the bias seed) to stay
        # within the 8 PSUM banks.
        ps3 = ps.tile([1, 2], f32)
        nc.tensor.matmul(ps3[0:1, 0:1], one1, one1, start=True, stop=True)

        # ---- r replicated on 16 PSUM partitions (feeds the ||r||^2 chain):
        # rp16f[p, n] = sum_k W^T[k, n] = r[n]   (f32 for an exact ||r||^2)
        rp16f = ps.tile([16, N], f32)
        with tc.high_priority():
            nc.tensor.matmul(rp16f, ones30x16, wt16, start=True, stop=True)

        # ---- QS16[4kh+kw, y] = qpad[y+kh] = r[y+kh-1] (row-shift Hankel)
        # and rp16h[4kh+kw, j] = rpad[j+kw] = r[j+kw-1] (col-shift Hankel).
        # PSUM matmul outs must start at partition 0, so each matmul writes
        # the full 16-lane tile with a one-hot lhsT block (only its lane
        # class lands nonzero) and they accumulate.  The shift-0 classes
        # need the qpad[0]=rpad[0]=0 edge: their matmuls cover cols 1: only;
        # col 0 is zero-initialized by the first matmul's start=True.
        QR16 = ps.tile([16, 48], f32)
        QS16 = QR16[:, 0:24]
        rp16h = QR16[:, 24:48]
        tc_hp = tc.high_priority()
        tc_hp.__enter__()
        for i, kh in enumerate([1, 2, 3]):
            nc.tensor.matmul(QS16, eselq[:, 16 * kh:16 * kh + 16],
                             bass.AP(wt16.tensor, wt16.offset + kh - 1,
                                     [list(wt16.ap[0]), [1, 24]]),
                             start=(i == 0), stop=False)
        nc.tensor.matmul(QS16[:, 1:24], eselq[:, 0:16],
                         bass.AP(wt16.tensor, wt16.offset,
                                 [list(wt16.ap[0]), [1, 23]]),
                         start=False, stop=True)
        for i, kw in enumerate([1, 2, 3]):
            nc.tensor.matmul(rp16h, eselr[:, 16 * kw:16 * kw + 16],
                             bass.AP(wt16.tensor, wt16.offset + kw - 1,
                                     [list(wt16.ap[0]), [1, 24]]),
                             start=(i == 0), stop=False)
        nc.tensor.matmul(rp16h[:, 1:24], eselr[:, 0:16],
                         bass.AP(wt16.tensor, wt16.offset,
                                 [list(wt16.ap[0]), [1, 23]]),
                         start=False, stop=True)
        tc_hp.__exit__(None, None, None)

        # ---- conv1 PSUM banks (one 8-row pool band each): static mask
        # channel + conv1 bias via one identity matmul per bank.
        banks = [ps.tile([8, 192], f32, name=f"ps1_{i}") for i in range(3)]
        for bi, pb in enumerate(banks):
            nc.tensor.matmul(pb, i8, static8[:, 192 * bi:192 * bi + 192],
                             start=True, stop=False)

        # Early: seed ps3 with the linear-layer bias (1x1 ones @ ob row).
        nc.tensor.matmul(ps3, one1, ob, start=True, stop=False)

        # ---- normalization: ||r||^2 per-partition on Activation (square +
        # per-partition accumulate over the 16 identical rows of rp16f, so
        # inv16 needs no cross-partition broadcast), reciprocal on DVE.
        # The 1/||r||^2 scale is folded into the image build itself (the
        # stt scalar operand), so no scaled-weight tensor is needed at all.
        sq16 = sb.tile([16, N], f32)
        ss16 = sb.tile([16, 1], f32)
        nc.scalar.activation(sq16, rp16f, mybir.ActivationFunctionType.Square,
                             accum_out=ss16)
        inv16 = sb.tile([16, 1], f32)

        # DVE ops with a PSUM input are limited to ONE PSUM operand, so QS
        # moves to SBUF first (cheap [16,24] copy); rp16h stays PSUM.
        qr_sb = sb.tile([16, 48], f32)

        # ---- the doubly-shifted rank-1 image, all 16 lanes in TWO
        # multi-partition DVE stt ops (normalization scalar fused), split
        # so bank 1's matmul starts while the rest is still building:
        # R[p, 32y+j] = QS16[p, y]/||r||^2 * rp16h[p, j],  y 0..23, j 0..23
        with tc.high_priority():
            nc.vector.tensor_copy(qr_sb, QR16)
            nc.vector.reciprocal(inv16, ss16)
            for y0, ny in ((0, 8), (8, 16)):
                img_out = bass.AP(R.tensor, R.offset + 32 * y0,
                                  [list(R.ap[0]), [32, ny], [1, 24]])
                img_q = bass.AP(qr_sb.tensor, qr_sb.offset + y0,
                                [list(qr_sb.ap[0]), [1, ny], [0, 24]])
                img_r = bass.AP(qr_sb.tensor, qr_sb.offset + 24,
                                [list(qr_sb.ap[0]), [0, ny], [1, 24]])
                nc.vector.scalar_tensor_tensor(img_out, img_q, inv16, img_r,
                                               ALU.mult, ALU.mult)

        # ---- conv1 rank-1 channel: ONE K=16 matmul per bank.
        pstride_R = R.ap[0][0]
        for bi, pb in enumerate(banks):
            rhs_b = bass.AP(R.tensor, R.offset + 256 * bi,
                            [[pstride_R, 16], [32, 8], [1, 24]])
            nc.tensor.matmul(pb, w1r, rhs_b, start=False, stop=True)

        # ---- maxpool 8x8 directly on PSUM: one DVE reduce per band,
        # pipelined behind the band matmuls; each band's leaky (into the
        # zero-padded 5x5 conv2 input) follows its reduce immediately so
        # only the last band's leaky trails the last reduce.
        pool1 = sb.tile([8, 9], f32)        # (pr, pc) row-major
        p1v = p1p.rearrange("p (h w) -> p h w", h=5)
        p13 = pool1.rearrange("p (h w) -> p h w", h=3)
        for bi, pb in enumerate(banks):
            vb = pb.rearrange("p (h pc w) -> p pc h w", h=8, pc=3)
            nc.vector.tensor_reduce(pool1[:, 3 * bi:3 * bi + 3], vb,
                                    axis=AX.XY, op=ALU.max)
        nc.vector.scalar_tensor_tensor(
            p1v[:, 1:4, 1:4], p13, 0.2, p13, ALU.mult, ALU.max)

        # ---- conv2: 8->16, k2, pad 1 -> (16, 4, 4)
        ps2 = ps.tile([16, 16], f32)
        w2v = w2l.rearrange("p (pos co) -> p pos co", pos=4)
        # bias seeded first (start=True, needs only wpack) so the last conv
        # matmul carries the stop and pooling starts one hop earlier
        nc.tensor.matmul(ps2, b2row, oner, start=True, stop=False)
        for kh in range(2):
            for kw in range(2):
                i = kh * 2 + kw
                nc.tensor.matmul(
                    ps2, w2v[:, i, :], p1v[:, kh:kh + 4, kw:kw + 4],
                    start=False, stop=(i == 3))

        # ---- maxpool 4x4 (whole map) from PSUM (bias in PSUM), then leaky
        hraw = sb.tile([16, 1], f32)
        nc.vector.tensor_reduce(hraw, ps2, axis=AX.X, op=ALU.max)
        hcol = sb.tile([16, 1], f32)
        nc.vector.scalar_tensor_tensor(hcol, hraw, 0.2, hraw, ALU.mult, ALU.max)

        # ---- linear 16 -> 2 (bias already seeded in ps3)
        nc.tensor.matmul(ps3, hcol, owt, start=False, stop=True)
        res = sb.tile([1, 2], f32)
        nc.vector.tensor_copy(res, ps3)

        nc.sync.dma_start(out=out_d.ap(), in_=res)

    nc.compile()
    return nc


def _get_nc():
    if "nc" not in _CACHE:
        _CACHE["nc"] = _build_nc()
    return _CACHE["nc"]


def make_in_map(W, conv1_w, conv1_b, conv2_w, conv2_b, out_w, out_b):
    W = np.asarray(W, np.float32)
    conv1_w = np.asarray(conv1_w, np.float32)
    conv1_b = np.asarray(conv1_b, np.float32)
    conv2_w = np.asarray(conv2_w, np.float32)
    wpack = np.zeros((16, 384), np.float32)

    # cols 0:4 -- rank-1 channel conv1 weights, pre-scaled by 0.5, f16,
    # laid out ((kh kw), co) = (16, 8) to match the 16 image lanes
    w1r_pre = 0.5 * conv1_w[:, 0]                    # (co, kh, kw)
    wpack[0:16, 0:4] = np.ascontiguousarray(np.asarray(
        w1r_pre.transpose(1, 2, 0).reshape(16, 8),
        np.float16)).view(np.float32)

    # cols 16:48 -- conv2 weights (ci, (pos co)), f16
    wpack[0:8, 16:48] = np.ascontiguousarray(np.asarray(
        conv2_w.transpose(1, 2, 3, 0).reshape(8, 64),
        np.float16)).view(np.float32)

    # cols 80:368 -- static mask-channel conv1 output + conv1 bias, f16:
    # static[co, y, x] = sum_{kh,kw} (w1[co,0]+w1[co,1])[kh,kw] *
    #                     0.5*maskpad[y+kh, x+kw] + b1[co],  y,x in 0..23
    w1sum = conv1_w.sum(axis=1)                      # (co, kh, kw) = (8,4,4)
    maskpad = np.zeros((33, 33), np.float32)
    maskpad[1:31, 1:31] = 0.5
    static = np.zeros((8, 24, 24), np.float32)
    for kh in range(4):
        for kw in range(4):
            static += w1sum[:, kh, kw][:, None, None] * \
                maskpad[kh:kh + 24, kw:kw + 24][None]
    static += conv1_b[:, None, None]
    # bank bi covers out rows 8bi..8bi+7 in layout (h, pc, w) = (8, 3, 8)
    st = static.reshape(8, 3, 8, 3, 8)               # (co, bh, h, pc, w)
    st = st.transpose(0, 1, 2, 3, 4).reshape(8, 576)  # (co, (bh h pc w))
    wpack[0:8, 80:368] = np.asarray(st, np.float16).reshape(8, 576).view(
        np.float32)

    # cols 368:372 -- 8x8 identity, f16 (lhsT of the static-injection matmul)
    wpack[0:8, 368:372] = np.asarray(np.eye(8), np.float16).view(np.float32)

    wpack[0:16, 372:374] = np.asarray(out_w, np.float32).T
    wpack[0, 374:376] = np.asarray(out_b, np.float32)
    wpack[0:1, 376:384] = np.asarray(
        conv2_b, np.float16).reshape(1, 16).view(np.float32)
    return {
        "wt": np.ascontiguousarray(np.asarray(W.T, np.float16)).view(
            np.float32),
        "wpack": wpack,
    }


def kernel(x=None, W=None, conv1_w=None, conv1_b=None, conv2_w=None,
           conv2_b=None, out_w=None, out_b=None, col=None, **_unused):
    from concourse.bass_utils import run_bass_kernel_spmd

    nc = _get_nc()
    in_map = make_in_map(W, conv1_w, conv1_b, conv2_w, conv2_b, out_w, out_b)
    n_cores = 8
    res = run_bass_kernel_spmd(nc, [in_map] * n_cores, core_ids=list(range(n_cores)))
    out = np.asarray(res.results[0]["out"], np.float32).reshape(1, 2)
    return out


# revision 97
# speedup vs baseline: 1.0044x; 1.0044x over previous
"""Trainium2 Bass kernel for nn_CNN2 (time-lagged cross-correlation CNN).

Math note (exact algebraic identity, not an approximation):
  The reference computes Y = W @ ones(30, T), so every time-column of Y is
  r = W.sum(axis=1).  The full lagged cross-correlation is then
  S[lag] = count(lag) * r r^T, its trace is count(lag) * ||r||^2, so the
  per-lag trace-normalized matrix S_mean[lag] = r r^T / ||r||^2 is the SAME
  for every lag.  Hence mean-over-lags = r r^T/||r||^2 and var-over-lags = 0
  for ANY W and ANY T.  The kernel therefore computes
      Gm = 0.5*(r r^T/||r||^2 + 1),  Gv = 0.5
  followed by the CNN tail (conv 2->8 k4 p1, leaky 0.2, maxpool 8;
  conv 8->16 k2 p1, leaky, maxpool 4; linear 16->2), all on-device.

Implementation notes (latency-driven; the kernel is dominated by fixed
DMA/semaphore latencies, so the design minimizes the serial depth between
the W input DMA and the output DMA):
  - The conv1 input channels split linearly: Gm = M + 0.5*mask and
    Gv = 0.5*mask with M = 0.5*r r^T/||r||^2.  The mask part of
    conv1 (+ conv1 bias) does not depend on W at all, so it is precomputed
    on the host and injected into the conv1 PSUM banks by one
    identity-weight matmul per bank.
  - r never leaves the chip, and W^T ships from the host directly in
    fp16 (the image is fp16 anyway, so this costs no precision and
    removes an on-chip convert).  rp16f = ones^T @ W^T replicates r on
    16 PSUM partitions for the ||r||^2 chain (Activation square-accum ->
    DVE reciprocal), and eight tiny one-hot-lhsT fp16 matmuls against
    shifted column windows of W^T build BOTH shift-Hankels in one PSUM
    tile: QS16[4kh+kw, y] = r[y+kh-1], rp16h[4kh+kw, j] = r[j+kw-1].
    One DVE copy moves them to SBUF and two multi-partition DVE stt ops
    (1/||r||^2 fused as the per-lane scalar) build all SIXTEEN
    doubly-shifted rank-1 image copies R[4kh+kw, 32y+j] =
    image[y+kh, j+kw], so each conv1 bank is ONE K=16 fp16 matmul.
    This replaces the SBUF->SBUF fan-out DMA of the first version
    (~2.2us of descriptor/semaphore latency) with ~1us of overlapped
    PE+DVE work.
  - conv1 output rows are split into three PSUM banks of one 8-row pool
    band each, so each band's maxpool reduce (DVE) pipelines behind the
    next band's matmuls.
  - maxpool commutes with x -> leaky(x+b) (monotone, per-channel b): conv2
    bias is accumulated into PSUM via a bias-row x ones-row matmul, pooling
    runs directly on PSUM, and only the leaky remains on the pooled tensors.
  - the linear-layer bias is pre-accumulated into the output PSUM early
    (ones(1,1) x bias-row matmul), so after the last matmul the result only
    needs a PSUM->SBUF copy before the output DMA.
  - conv1 runs in fp16 (1 PE cycle/row; checked end-to-end error ~3e-4).
    A tiny ones x ones matmul with no input dependencies runs right after
    the entry barrier to start the PE clock-ramp tracker early, so the
    critical matmuls run at the ramped clock rate.

The computation is replicated on the 8 NeuronCores (it is far below one
core's capacity; a cross-core split would only add collective latency), and
core 0's output is returned.
"""

import numpy as np

N = 30

_CACHE = {}


def _build_nc():
    from contextlib import ExitStack

    import concourse.bass as bass
    import concourse.tile as tile
    from concourse import bacc, mybir

    f32 = mybir.dt.float32
    f16 = mybir.dt.float16
    ALU = mybir.AluOpType
    AX = mybir.AxisListType

    nc = bacc.Bacc("TRN2")

    wt_d = nc.dram_tensor("wt", [N, 15], f32, kind="ExternalInput")     # W^T f16
    wp_d = nc.dram_tensor("wpack", [16, 384], f32, kind="ExternalInput")
    out_d = nc.dram_tensor("out", [1, 2], f32, kind="ExternalOutput")

    with tile.TileContext(nc) as tc, ExitStack() as ctx:
        sb = ctx.enter_context(tc.tile_pool(name="sb", bufs=1))
        ps = ctx.enter_context(tc.tile_pool(name="ps", bufs=1, space="PSUM"))

        # Pool memsets, in priority order (the first ones get hoisted ahead
        # of the entry barrier): one1 feeds the PE warm-up matmul.
        one1 = sb.tile([1, 1], f32)
        nc.gpsimd.memset(one1, 1.0)
        wpack = sb.tile([16, 384], f32)
        nc.gpsimd.dma_start(out=wpack, in_=wp_d.ap())
        ones30x16 = sb.tile([N, 16], f16)
        nc.gpsimd.memset(ones30x16, 1.0)
        # One-hot selector lhsT blocks.  The 16 image lanes are indexed
        # p = 4*kh + kw.  eselq block kh (cols 16kh..16kh+15) has ones at
        # within-block cols 4kh+t (t=0..3) -> global cols 20kh+t: the kh-th
        # QS16 matmul writes lanes 4kh..4kh+3.  eselr block kw has ones at
        # within-block cols kw+4t -> global cols 17kw+4t: the kw-th rp16
        # matmul writes lanes kw, kw+4, kw+8, kw+12.
        eselq = sb.tile([N, 64], f16)
        nc.gpsimd.memset(eselq, 0.0)
        nc.gpsimd.memset(bass.AP(eselq.tensor, eselq.offset,
                                 [list(eselq.ap[0]), [20, 4], [1, 4]]), 1.0)
        eselr = sb.tile([N, 64], f16)
        nc.gpsimd.memset(eselr, 0.0)
        nc.gpsimd.memset(bass.AP(eselr.tensor, eselr.offset,
                                 [list(eselr.ap[0]), [17, 4], [4, 4]]), 1.0)

        wt = sb.tile([N, 15], f32)
        nc.sync.dma_start(out=wt, in_=wt_d.ap())
        wt16 = wt.bitcast(f16)                 # (30, 30) f16 W^T view

        w1r = wpack[0:16, 0:4].bitcast(f16)    # (16, 8): 0.5*w1[:,0] (kh kw, co)
        w2l = wpack[0:8, 16:48].bitcast(f16)   # conv2 weights (ci, (pos co))
        static8 = wpack[0:8, 80:368].bitcast(f16)   # (8, 576) mask-ch conv out
        i8 = wpack[0:8, 368:372].bitcast(f16)       # (8, 8) identity
        owt = wpack[0:16, 372:374]
        ob = wpack[0:1, 374:376]
        b2row = wpack[0:1, 376:384].bitcast(f16)    # (1, 16) f16

        # fp16 conv rhs tile: R[4kh+kw, 32y+j] = image[y+kh, j+kw] for
        # j in 0..23 -- each lane is the image shifted in BOTH kh and kw,
        # so conv1's rank-1 channel is ONE K=16 matmul per bank.  Columns
        # 24..31 of each 32-block are never read, so no border memset.
        R = sb.tile([16, 768], f16)
        oner = sb.tile([1, 16], f16)
        nc.gpsimd.memset(oner, 1.0)
        p1p = sb.tile([8, 25], f16)
        nc.gpsimd.memset(p1p, 0.0)

        # PE warm-up: no input deps, starts the clock-ramp tracker early.
        # It borrows ps3 (overwritten below by the bias seed) to stay
        # within the 8 PSUM banks.
        ps3 = ps.tile([1, 2], f32)
        nc.tensor.matmul(ps3[0:1, 0:1], one1, one1, start=True, stop=True)

        # ---- r replicated on 16 PSUM partitions (feeds the ||r||^2 chain):
        # rp16f[p, n] = sum_k W^T[k, n] = r[n]   (f32 for an exact ||r||^2)
        rp16f = ps.tile([16, N], f32)
        with tc.high_priority():
            nc.tensor.matmul(rp16f, ones30x16, wt16, start=True, stop=True)

        # ---- QS16[4kh+kw, y] = qpad[y+kh] = r[y+kh-1] (row-shift Hankel)
        # and rp16h[4kh+kw, j] = rpad[j+kw] = r[j+kw-1] (col-shift Hankel).
        # PSUM matmul outs must start at partition 0, so each matmul writes
        # the full 16-lane tile with a one-hot lhsT block (only its lane
        # class lands nonzero) and they accumulate.  The shift-0 classes
        # need the qpad[0]=rpad[0]=0 edge: their matmuls cover cols 1: only;
        # col 0 is zero-initialized by the first matmul's start=True.
        QR16 = ps.tile([16, 48], f32)
        QS16 = QR16[:, 0:24]
        rp16h = QR16[:, 24:48]
        tc_hp = tc.high_priority()
        tc_hp.__enter__()
        for i, kh in enumerate([1, 2, 3]):
            nc.tensor.matmul(QS16, eselq[:, 16 * kh:16 * kh + 16],
                             bass.AP(wt16.tensor, wt16.offset + kh - 1,
                                     [list(wt16.ap[0]), [1, 24]]),
                             start=(i == 0), stop=False)
        nc.tensor.matmul(QS16[:, 1:24], eselq[:, 0:16],
                         bass.AP(wt16.tensor, wt16.offset,
                                 [list(wt16.ap[0]), [1, 23]]),
                         start=False, stop=True)
        for i, kw in enumerate([1, 2, 3]):
            nc.tensor.matmul(rp16h, eselr[:, 16 * kw:16 * kw + 16],
                             bass.AP(wt16.tensor, wt16.offset + kw - 1,
                                     [list(wt16.ap[0]), [1, 24]]),
                             start=(i == 0), stop=False)
        nc.tensor.matmul(rp16h[:, 1:24], eselr[:, 0:16],
                         bass.AP(wt16.tensor, wt16.offset,
                                 [list(wt16.ap[0]), [1, 23]]),
                         start=False, stop=True)
        tc_hp.__exit__(None, None, None)

        # ---- conv1 PSUM banks (one 8-row pool band each): static mask
        # channel + conv1 bias via one identity matmul per bank.
        banks = [ps.tile([8, 192], f32, name=f"ps1_{i}") for i in range(3)]
        for bi, pb in enumerate(banks):
            nc.tensor.matmul(pb, i8, static8[:, 192 * bi:192 * bi + 192],
                             start=True, stop=False)

        # Early: seed ps3 with the linear-layer bias (1x1 ones @ ob row).
        nc.tensor.matmul(ps3, one1, ob, start=True, stop=False)

        # ---- normalization: ||r||^2 per-partition on Activation (square +
        # per-partition accumulate over the 16 identical rows of rp16f, so
        # inv16 needs no cross-partition broadcast), reciprocal on DVE.
        # The 1/||r||^2 scale is folded into the image build itself (the
        # stt scalar operand), so no scaled-weight tensor is needed at all.
        sq16 = sb.tile([16, N], f32)
        ss16 = sb.tile([16, 1], f32)
        nc.scalar.activation(sq16, rp16f, mybir.ActivationFunctionType.Square,
                             accum_out=ss16)
        inv16 = sb.tile([16, 1], f32)

        # DVE ops with a PSUM input are limited to ONE PSUM operand, so QS
        # moves to SBUF first (cheap [16,24] copy); rp16h stays PSUM.
        qr_sb = sb.tile([16, 48], f32)

        # ---- the doubly-shifted rank-1 image, all 16 lanes in TWO
        # multi-partition DVE stt ops (normalization scalar fused), split
        # so bank 1's matmul starts while the rest is still building:
        # R[p, 32y+j] = QS16[p, y]/||r||^2 * rp16h[p, j],  y 0..23, j 0..23
        with tc.high_priority():
            nc.vector.tensor_copy(qr_sb, QR16)
            nc.vector.reciprocal(inv16, ss16)
            for y0, ny in ((0, 8), (8, 16)):
                img_out = bass.AP(R.tensor, R.offset + 32 * y0,
                                  [list(R.ap[0]), [32, ny], [1, 24]])
                img_q = bass.AP(qr_sb.tensor, qr_sb.offset + y0,
                                [list(qr_sb.ap[0]), [1, ny], [0, 24]])
                img_r = bass.AP(qr_sb.tensor, qr_sb.offset + 24,
                                [list(qr_sb.ap[0]), [0, ny], [1, 24]])
                nc.vector.scalar_tensor_tensor(img_out, img_q, inv16, img_r,
                                               ALU.mult, ALU.mult)

        # ---- conv1 rank-1 channel: ONE K=16 matmul per bank.
        pstride_R = R.ap[0][0]
        for bi, pb in enumerate(banks):
            rhs_b = bass.AP(R.tensor, R.offset + 256 * bi,
                            [[pstride_R, 16], [32, 8], [1, 24]])
            nc.tensor.matmul(pb, w1r, rhs_b, start=False, stop=True)

        # ---- maxpool 8x8 directly on PSUM: one DVE reduce per band,
        # pipelined behind the band matmuls; each band's leaky (into the
        # zero-padded 5x5 conv2 input) follows its reduce immediately so
        # only the last band's leaky trails the last reduce.
        pool1 = sb.tile([8, 9], f32)        # (pr, pc) row-major
        p1v = p1p.rearrange("p (h w) -> p h w", h=5)
        p13 = pool1.rearrange("p (h w) -> p h w", h=3)
        for bi, pb in enumerate(banks):
            vb = pb.rearrange("p (h pc w) -> p pc h w", h=8, pc=3)
            nc.vector.tensor_reduce(pool1[:, 3 * bi:3 * bi + 3], vb,
                                    axis=AX.XY, op=ALU.max)
            nc.vector.scalar_tensor_tensor(
                p1v[:, bi + 1:bi + 2, 1:4], p13[:, bi:bi + 1, :], 0.2,
                p13[:, bi:bi + 1, :], ALU.mult, ALU.max)

        # ---- conv2: 8->16, k2, pad 1 -> (16, 4, 4)
        ps2 = ps.tile([16, 16], f32)
        w2v = w2l.rearrange("p (pos co) -> p pos co", pos=4)
        # bias seeded first (start=True, needs only wpack) so the last conv
        # matmul carries the stop and pooling starts one hop earlier
        nc.tensor.matmul(ps2, b2row, oner, start=True, stop=False)
        for kh in range(2):
            for kw in range(2):
                i = kh * 2 + kw
                nc.tensor.matmul(
                    ps2, w2v[:, i, :], p1v[:, kh:kh + 4, kw:kw + 4],
                    start=False, stop=(i == 3))

        # ---- maxpool 4x4 (whole map) from PSUM (bias in PSUM), then leaky
        hraw = sb.tile([16, 1], f32)
        nc.vector.tensor_reduce(hraw, ps2, axis=AX.X, op=ALU.max)
        hcol = sb.tile([16, 1], f32)
        nc.vector.scalar_tensor_tensor(hcol, hraw, 0.2, hraw, ALU.mult, ALU.max)

        # ---- linear 16 -> 2 (bias already seeded in ps3)
        nc.tensor.matmul(ps3, hcol, owt, start=False, stop=True)
        res = sb.tile([1, 2], f32)
        nc.vector.tensor_copy(res, ps3)

        nc.sync.dma_start(out=out_d.ap(), in_=res)

    nc.compile()
    return nc


def _get_nc():
    if "nc" not in _CACHE:
        _CACHE["nc"] = _build_nc()
    return _CACHE["nc"]


def make_in_map(W, conv1_w, conv1_b, conv2_w, conv2_b, out_w, out_b):
    W = np.asarray(W, np.float32)
    conv1_w = np.asarray(conv1_w, np.float32)
    conv1_b = np.asarray(conv1_b, np.float32)
    conv2_w = np.asarray(conv2_w, np.float32)
    wpack = np.zeros((16, 384), np.float32)

    # cols 0:4 -- rank-1 channel conv1 weights, pre-scaled by 0.5, f16,
    # laid out ((kh kw), co) = (16, 8) to match the 16 image lanes
    w1r_pre = 0.5 * conv1_w[:, 0]                    # (co, kh, kw)
    wpack[0:16, 0:4] = np.ascontiguousarray(np.asarray(
        w1r_pre.transpose(1, 2, 0).reshape(16, 8),
        np.float16)).view(np.float32)

    # cols 16:48 -- conv2 weights (ci, (pos co)), f16
    wpack[0:8, 16:48] = np.ascontiguousarray(np.asarray(
        conv2_w.transpose(1, 2, 3, 0).reshape(8, 64),
        np.float16)).view(np.float32)

    # cols 80:368 -- static mask-channel conv1 output + conv1 bias, f16:
    # static[co, y, x] = sum_{kh,kw} (w1[co,0]+w1[co,1])[kh,kw] *
    #                     0.5*maskpad[y+kh, x+kw] + b1[co],  y,x in 0..23
    w1sum = conv1_w.sum(axis=1)                      # (co, kh, kw) = (8,4,4)
    maskpad = np.zeros((33, 33), np.float32)
    maskpad[1:31, 1:31] = 0.5
    static = np.zeros((8, 24, 24), np.float32)
    for kh in range(4):
        for kw in range(4):
            static += w1sum[:, kh, kw][:, None, None] * \
                maskpad[kh:kh + 24, kw:kw + 24][None]
    static += conv1_b[:, None, None]
    # bank bi covers out rows 8bi..8bi+7 in layout (h, pc, w) = (8, 3, 8)
    st = static.reshape(8, 3, 8, 3, 8)               # (co, bh, h, pc, w)
    st = st.transpose(0, 1, 2, 3, 4).reshape(8, 576)  # (co, (bh h pc w))
    wpack[0:8, 80:368] = np.asarray(st, np.float16).reshape(8, 576).view(
        np.float32)

    # cols 368:372 -- 8x8 identity, f16 (lhsT of the static-injection matmul)
    wpack[0:8, 368:372] = np.asarray(np.eye(8), np.float16).view(np.float32)

    wpack[0:16, 372:374] = np.asarray(out_w, np.float32).T
    wpack[0, 374:376] = np.asarray(out_b, np.float32)
    wpack[0:1, 376:384] = np.asarray(
        conv2_b, np.float16).reshape(1, 16).view(np.float32)
    return {
        "wt": np.ascontiguousarray(np.asarray(W.T, np.float16)).view(
            np.float32),
        "wpack": wpack,
    }


def kernel(x=None, W=None, conv1_w=None, conv1_b=None, conv2_w=None,
           conv2_b=None, out_w=None, out_b=None, col=None, **_unused):
    from concourse.bass_utils import run_bass_kernel_spmd

    nc = _get_nc()
    in_map = make_in_map(W, conv1_w, conv1_b, conv2_w, conv2_b, out_w, out_b)
    n_cores = 8
    res = run_bass_kernel_spmd(nc, [in_map] * n_cores, core_ids=list(range(n_cores)))
    out = np.asarray(res.results[0]["out"], np.float32).reshape(1, 2)
    return out


# revision 98
# speedup vs baseline: 1.0112x; 1.0068x over previous
"""Trainium2 Bass kernel for nn_CNN2 (time-lagged cross-correlation CNN).

Math note (exact algebraic identity, not an approximation):
  The reference computes Y = W @ ones(30, T), so every time-column of Y is
  r = W.sum(axis=1).  The full lagged cross-correlation is then
  S[lag] = count(lag) * r r^T, its trace is count(lag) * ||r||^2, so the
  per-lag trace-normalized matrix S_mean[lag] = r r^T / ||r||^2 is the SAME
  for every lag.  Hence mean-over-lags = r r^T/||r||^2 and var-over-lags = 0
  for ANY W and ANY T.  The kernel therefore computes
      Gm = 0.5*(r r^T/||r||^2 + 1),  Gv = 0.5
  followed by the CNN tail (conv 2->8 k4 p1, leaky 0.2, maxpool 8;
  conv 8->16 k2 p1, leaky, maxpool 4; linear 16->2), all on-device.

Implementation notes (latency-driven; the kernel is dominated by fixed
DMA/semaphore latencies, so the design minimizes the serial depth between
the W input DMA and the output DMA):
  - The conv1 input channels split linearly: Gm = M + 0.5*mask and
    Gv = 0.5*mask with M = 0.5*r r^T/||r||^2.  The mask part of
    conv1 (+ conv1 bias) does not depend on W at all, so it is precomputed
    on the host and injected into the conv1 PSUM banks by one
    identity-weight matmul per bank.
  - r never leaves the chip, and W^T ships from the host directly in
    fp16 (the image is fp16 anyway, so this costs no precision and
    removes an on-chip convert).  rp16f = ones^T @ W^T replicates r on
    16 PSUM partitions for the ||r||^2 chain (Activation square-accum ->
    DVE reciprocal), and eight tiny one-hot-lhsT fp16 matmuls against
    shifted column windows of W^T build BOTH shift-Hankels in one PSUM
    tile: QS16[4kh+kw, y] = r[y+kh-1], rp16h[4kh+kw, j] = r[j+kw-1].
    One DVE copy moves them to SBUF and two multi-partition DVE stt ops
    (1/||r||^2 fused as the per-lane scalar) build all SIXTEEN
    doubly-shifted rank-1 image copies R[4kh+kw, 32y+j] =
    image[y+kh, j+kw], so each conv1 bank is ONE K=16 fp16 matmul.
    This replaces the SBUF->SBUF fan-out DMA of the first version
    (~2.2us of descriptor/semaphore latency) with ~1us of overlapped
    PE+DVE work.
  - conv1 output rows are split into three PSUM banks of one 8-row pool
    band each, so each band's maxpool reduce (DVE) pipelines behind the
    next band's matmuls.
  - maxpool commutes with x -> leaky(x+b) (monotone, per-channel b): conv2
    bias is accumulated into PSUM via a bias-row x ones-row matmul, pooling
    runs directly on PSUM, and only the leaky remains on the pooled tensors.
  - the linear-layer bias is pre-accumulated into the output PSUM early
    (ones(1,1) x bias-row matmul), so after the last matmul the result only
    needs a PSUM->SBUF copy before the output DMA.
  - conv1 runs in fp16 (1 PE cycle/row; checked end-to-end error ~3e-4).
    A tiny ones x ones matmul with no input dependencies runs right after
    the entry barrier to start the PE clock-ramp tracker early, so the
    critical matmuls run at the ramped clock rate.

The computation is replicated on the 8 NeuronCores (it is far below one
core's capacity; a cross-core split would only add collective latency), and
core 0's output is returned.
"""

import numpy as np

N = 30

_CACHE = {}


def _build_nc():
    from contextlib import ExitStack

    import concourse.bass as bass
    import concourse.tile as tile
    from concourse import bacc, mybir

    f32 = mybir.dt.float32
    f16 = mybir.dt.float16
    ALU = mybir.AluOpType
    AX = mybir.AxisListType

    nc = bacc.Bacc("TRN2")

    wt_d = nc.dram_tensor("wt", [N, 15], f32, kind="ExternalInput")     # W^T f16
    wp_d = nc.dram_tensor("wpack", [16, 384], f32, kind="ExternalInput")
    out_d = nc.dram_tensor("out", [1, 2], f32, kind="ExternalOutput")

    with tile.TileContext(nc) as tc, ExitStack() as ctx:
        sb = ctx.enter_context(tc.tile_pool(name="sb", bufs=1))
        ps = ctx.enter_context(tc.tile_pool(name="ps", bufs=1, space="PSUM"))

        # Pool memsets, in priority order (the first ones get hoisted ahead
        # of the entry barrier): one1 feeds the PE warm-up matmul.
        one1 = sb.tile([1, 1], f32)
        nc.gpsimd.memset(one1, 1.0)
        wpack = sb.tile([16, 384], f32)
        nc.gpsimd.dma_start(out=wpack, in_=wp_d.ap())
        ones30x16 = sb.tile([N, 16], f16)
        nc.gpsimd.memset(ones30x16, 1.0)
        # One-hot selector lhsT blocks.  The 16 image lanes are indexed
        # p = 4*kh + kw.  eselq block kh (cols 16kh..16kh+15) has ones at
        # within-block cols 4kh+t (t=0..3) -> global cols 20kh+t: the kh-th
        # QS16 matmul writes lanes 4kh..4kh+3.  eselr block kw has ones at
        # within-block cols kw+4t -> global cols 17kw+4t: the kw-th rp16
        # matmul writes lanes kw, kw+4, kw+8, kw+12.
        eselq = sb.tile([N, 64], f16)
        nc.gpsimd.memset(eselq, 0.0)
        nc.gpsimd.memset(bass.AP(eselq.tensor, eselq.offset,
                                 [list(eselq.ap[0]), [20, 4], [1, 4]]), 1.0)
        eselr = sb.tile([N, 64], f16)
        nc.gpsimd.memset(eselr, 0.0)
        nc.gpsimd.memset(bass.AP(eselr.tensor, eselr.offset,
                                 [list(eselr.ap[0]), [17, 4], [4, 4]]), 1.0)

        wt = sb.tile([N, 15], f32)
        nc.sync.dma_start(out=wt, in_=wt_d.ap())
        wt16 = wt.bitcast(f16)                 # (30, 30) f16 W^T view

        w1r = wpack[0:16, 0:4].bitcast(f16)    # (16, 8): 0.5*w1[:,0] (kh kw, co)
        w2l = wpack[0:8, 16:48].bitcast(f16)   # conv2 weights (ci, (pos co))
        static8 = wpack[0:8, 80:368].bitcast(f16)   # (8, 576) mask-ch conv out
        i8 = wpack[0:8, 368:372].bitcast(f16)       # (8, 8) identity
        owt = wpack[0:16, 372:374]
        ob = wpack[0:1, 374:376]
        b2row = wpack[0:1, 376:384].bitcast(f16)    # (1, 16) f16

        # fp16 conv rhs tile: R[4kh+kw, 32y+j] = image[y+kh, j+kw] for
        # j in 0..23 -- each lane is the image shifted in BOTH kh and kw,
        # so conv1's rank-1 channel is ONE K=16 matmul per bank.  Columns
        # 24..31 of each 32-block are never read, so no border memset.
        R = sb.tile([16, 768], f16)
        oner = sb.tile([1, 16], f16)
        nc.gpsimd.memset(oner, 1.0)
        p1p = sb.tile([8, 25], f16)
        nc.gpsimd.memset(p1p, 0.0)

        # PE warm-up: no input deps, starts the clock-ramp tracker early.
        # It borrows ps3 (overwritten below by the bias seed) to stay
        # within the 8 PSUM banks.
        ps3 = ps.tile([1, 2], f32)
        nc.tensor.matmul(ps3[0:1, 0:1], one1, one1, start=True, stop=True)

        # ---- r replicated on 16 PSUM partitions (feeds the ||r||^2 chain):
        # rp16f[p, n] = sum_k W^T[k, n] = r[n]   (f32 for an exact ||r||^2)
        rp16f = ps.tile([16, N], f32)
        with tc.high_priority():
            nc.tensor.matmul(rp16f, ones30x16, wt16, start=True, stop=True)

        # ---- QS16[4kh+kw, y] = qpad[y+kh] = r[y+kh-1] (row-shift Hankel)
        # and rp16h[4kh+kw, j] = rpad[j+kw] = r[j+kw-1] (col-shift Hankel).
        # PSUM matmul outs must start at partition 0, so each matmul writes
        # the full 16-lane tile with a one-hot lhsT block (only its lane
        # class lands nonzero) and they accumulate.  The shift-0 classes
        # need the qpad[0]=rpad[0]=0 edge: their matmuls cover cols 1: only;
        # col 0 is zero-initialized by the first matmul's start=True.
        QR16 = ps.tile([16, 48], f32)
        QS16 = QR16[:, 0:24]
        rp16h = QR16[:, 24:48]
        tc_hp = tc.high_priority()
        tc_hp.__enter__()
        for i, kh in enumerate([1, 2, 3]):
            nc.tensor.matmul(QS16, eselq[:, 16 * kh:16 * kh + 16],
                             bass.AP(wt16.tensor, wt16.offset + kh - 1,
                                     [list(wt16.ap[0]), [1, 24]]),
                             start=(i == 0), stop=False)
        nc.tensor.matmul(QS16[:, 1:24], eselq[:, 0:16],
                         bass.AP(wt16.tensor, wt16.offset,
                                 [list(wt16.ap[0]), [1, 23]]),
                         start=False, stop=True)
        for i, kw in enumerate([1, 2, 3]):
            nc.tensor.matmul(rp16h, eselr[:, 16 * kw:16 * kw + 16],
                             bass.AP(wt16.tensor, wt16.offset + kw - 1,
                                     [list(wt16.ap[0]), [1, 24]]),
                             start=(i == 0), stop=False)
        nc.tensor.matmul(rp16h[:, 1:24], eselr[:, 0:16],
                         bass.AP(wt16.tensor, wt16.offset,
                                 [list(wt16.ap[0]), [1, 23]]),
                         start=False, stop=True)
        tc_hp.__exit__(None, None, None)

        # ---- conv1 PSUM banks (one 8-row pool band each): static mask
        # channel + conv1 bias via one identity matmul per bank.
        banks = [ps.tile([8, 192], f32, name=f"ps1_{i}") for i in range(3)]
        for bi, pb in enumerate(banks):
            nc.tensor.matmul(pb, i8, static8[:, 192 * bi:192 * bi + 192],
                             start=True, stop=False)

        # Early: seed ps3 with the linear-layer bias (1x1 ones @ ob row).
        nc.tensor.matmul(ps3, one1, ob, start=True, stop=False)

        # ---- normalization: ||r||^2 per-partition on Activation (square +
        # per-partition accumulate over the 16 identical rows of rp16f, so
        # inv16 needs no cross-partition broadcast), reciprocal on DVE.
        # The 1/||r||^2 scale is folded into the image build itself (the
        # stt scalar operand), so no scaled-weight tensor is needed at all.
        sq16 = sb.tile([16, N], f32)
        ss16 = sb.tile([16, 1], f32)
        nc.scalar.activation(sq16, rp16f, mybir.ActivationFunctionType.Square,
                             accum_out=ss16)
        inv16 = sb.tile([16, 1], f32)

        # DVE ops with a PSUM input are limited to ONE PSUM operand, so QS
        # moves to SBUF first (cheap [16,24] copy); rp16h stays PSUM.
        qr_sb = sb.tile([16, 48], f32)

        # ---- the doubly-shifted rank-1 image, all 16 lanes in TWO
        # multi-partition DVE stt ops (normalization scalar fused), split
        # so bank 1's matmul starts while the rest is still building:
        # R[p, 32y+j] = QS16[p, y]/||r||^2 * rp16h[p, j],  y 0..23, j 0..23
        with tc.high_priority():
            nc.vector.tensor_copy(qr_sb, QR16)
            nc.vector.reciprocal(inv16, ss16)
            for y0, ny in ((0, 8), (8, 16)):
                img_out = bass.AP(R.tensor, R.offset + 32 * y0,
                                  [list(R.ap[0]), [32, ny], [1, 24]])
                img_q = bass.AP(qr_sb.tensor, qr_sb.offset + y0,
                                [list(qr_sb.ap[0]), [1, ny], [0, 24]])
                img_r = bass.AP(qr_sb.tensor, qr_sb.offset + 24,
                                [list(qr_sb.ap[0]), [0, ny], [1, 24]])
                nc.vector.scalar_tensor_tensor(img_out, img_q, inv16, img_r,
                                               ALU.mult, ALU.mult)

        # ---- conv1 rank-1 channel: ONE K=16 matmul per bank.
        pstride_R = R.ap[0][0]
        for bi, pb in enumerate(banks):
            rhs_b = bass.AP(R.tensor, R.offset + 256 * bi,
                            [[pstride_R, 16], [32, 8], [1, 24]])
            nc.tensor.matmul(pb, w1r, rhs_b, start=False, stop=True)

        # ---- maxpool 8x8 directly on PSUM: one DVE reduce per band,
        # pipelined behind the band matmuls; each band's leaky (into the
        # zero-padded 5x5 conv2 input) follows its reduce immediately so
        # only the last band's leaky trails the last reduce.
        pool1 = sb.tile([8, 9], f32)        # (pr, pc) row-major
        p1v = p1p.rearrange("p (h w) -> p h w", h=5)
        p13 = pool1.rearrange("p (h w) -> p h w", h=3)
        for bi, pb in enumerate(banks):
            vb = pb.rearrange("p (h pc w) -> p pc h w", h=8, pc=3)
            nc.vector.tensor_reduce(pool1[:, 3 * bi:3 * bi + 3], vb,
                                    axis=AX.XY, op=ALU.max)
        for bi in range(3):
            nc.vector.scalar_tensor_tensor(
                p1v[:, bi + 1:bi + 2, 1:4], p13[:, bi:bi + 1, :], 0.2,
                p13[:, bi:bi + 1, :], ALU.mult, ALU.max)

        # ---- conv2: 8->16, k2, pad 1 -> (16, 4, 4)
        ps2 = ps.tile([16, 16], f32)
        w2v = w2l.rearrange("p (pos co) -> p pos co", pos=4)
        # bias seeded first (start=True, needs only wpack) so the last conv
        # matmul carries the stop and pooling starts one hop earlier
        nc.tensor.matmul(ps2, b2row, oner, start=True, stop=False)
        for kh in range(2):
            for kw in range(2):
                i = kh * 2 + kw
                nc.tensor.matmul(
                    ps2, w2v[:, i, :], p1v[:, kh:kh + 4, kw:kw + 4],
                    start=False, stop=(i == 3))

        # ---- maxpool 4x4 (whole map) from PSUM (bias in PSUM), then leaky
        hraw = sb.tile([16, 1], f32)
        nc.vector.tensor_reduce(hraw, ps2, axis=AX.X, op=ALU.max)
        hcol = sb.tile([16, 1], f32)
        nc.vector.scalar_tensor_tensor(hcol, hraw, 0.2, hraw, ALU.mult, ALU.max)

        # ---- linear 16 -> 2 (bias already seeded in ps3)
        nc.tensor.matmul(ps3, hcol, owt, start=False, stop=True)
        res = sb.tile([1, 2], f32)
        nc.vector.tensor_copy(res, ps3)

        nc.sync.dma_start(out=out_d.ap(), in_=res)

    nc.compile()
    return nc


def _get_nc():
    if "nc" not in _CACHE:
        _CACHE["nc"] = _build_nc()
    return _CACHE["nc"]


def make_in_map(W, conv1_w, conv1_b, conv2_w, conv2_b, out_w, out_b):
    W = np.asarray(W, np.float32)
    conv1_w = np.asarray(conv1_w, np.float32)
    conv1_b = np.asarray(conv1_b, np.float32)
    conv2_w = np.asarray(conv2_w, np.float32)
    wpack = np.zeros((16, 384), np.float32)

    # cols 0:4 -- rank-1 channel conv1 weights, pre-scaled by 0.5, f16,
    # laid out ((kh kw), co) = (16, 8) to match the 16 image lanes
    w1r_pre = 0.5 * conv1_w[:, 0]                    # (co, kh, kw)
    wpack[0:16, 0:4] = np.ascontiguousarray(np.asarray(
        w1r_pre.transpose(1, 2, 0).reshape(16, 8),
        np.float16)).view(np.float32)

    # cols 16:48 -- conv2 weights (ci, (pos co)), f16
    wpack[0:8, 16:48] = np.ascontiguousarray(np.asarray(
        conv2_w.transpose(1, 2, 3, 0).reshape(8, 64),
        np.float16)).view(np.float32)

    # cols 80:368 -- static mask-channel conv1 output + conv1 bias, f16:
    # static[co, y, x] = sum_{kh,kw} (w1[co,0]+w1[co,1])[kh,kw] *
    #                     0.5*maskpad[y+kh, x+kw] + b1[co],  y,x in 0..23
    w1sum = conv1_w.sum(axis=1)                      # (co, kh, kw) = (8,4,4)
    maskpad = np.zeros((33, 33), np.float32)
    maskpad[1:31, 1:31] = 0.5
    static = np.zeros((8, 24, 24), np.float32)
    for kh in range(4):
        for kw in range(4):
            static += w1sum[:, kh, kw][:, None, None] * \
                maskpad[kh:kh + 24, kw:kw + 24][None]
    static += conv1_b[:, None, None]
    # bank bi covers out rows 8bi..8bi+7 in layout (h, pc, w) = (8, 3, 8)
    st = static.reshape(8, 3, 8, 3, 8)               # (co, bh, h, pc, w)
    st = st.transpose(0, 1, 2, 3, 4).reshape(8, 576)  # (co, (bh h pc w))
    wpack[0:8, 80:368] = np.asarray(st, np.float16).reshape(8, 576).view(
        np.float32)

    # cols 368:372 -- 8x8 identity, f16 (lhsT of the static-injection matmul)
    wpack[0:8, 368:372] = np.asarray(np.eye(8), np.float16).view(np.float32)

    wpack[0:16, 372:374] = np.asarray(out_w, np.float32).T
    wpack[0, 374:376] = np.asarray(out_b, np.float32)
    wpack[0:1, 376:384] = np.asarray(
        conv2_b, np.float16).reshape(1, 16).view(np.float32)
    return {
        "wt": np.ascontiguousarray(np.asarray(W.T, np.float16)).view(
            np.float32),
        "wpack": wpack,
    }


def kernel(x=None, W=None, conv1_w=None, conv1_b=None, conv2_w=None,
           conv2_b=None, out_w=None, out_b=None, col=None, **_unused):
    from concourse.bass_utils import run_bass_kernel_spmd

    nc = _get_nc()
    in_map = make_in_map(W, conv1_w, conv1_b, conv2_w, conv2_b, out_w, out_b)
    n_cores = 8
    res = run_bass_kernel_spmd(nc, [in_map] * n_cores, core_ids=list(range(n_cores)))
    out = np.asarray(res.results[0]["out"], np.float32).reshape(1, 2)
    return out
